# revision 1
# baseline (speedup 1.0000x reference)
"""3-layer Elman RNN (tanh) Trainium2 kernel.

Model: x(512,2048) int -> emb(27,20) lookup -> RNN 20->32 -> 32->64 -> 64->64
       -> FC 64->26.  Output (512, 2048, 26) f32.

Strategy (per core, batch sharded 8 ways -> 256 batch/core, split into two
ping-pong halves of 128 so ACT and PE overlap across the serial recurrence):

All three layers advance in a skewed pipeline: at macro-step s, layer 1
processes t=s, layer 2 t=s-1, layer 3 t=s-2.  Per half-step one PSUM tile
P[128, 256] holds all three pre-activations:
  P[0:64,  0:128]   = pre2     P[64:128, 0:128] = pre3
  P[64:96, 128:256] = pre1     (rest primed to zero once)
filled by 4 matmuls, then ONE ACT tanh op covers the whole tile; layer-2/3
biases ride the ACT per-partition bias vector, layer-1's bias is folded into
the one-hot embedding table (one-hot rows sum to 1).  pre1/h1 live on
partitions 64-95 so the three small matmuls occupy disjoint 32x32 subarray
rectangles (tile_position) and overlap on the PE; the one-hot matmul has no
dependence on the recurrence and fills the PE's ACT-wait window.  FC runs in
bulk per 2-step chunk with its bias via a constant-ones row; output is
written [26, T*B] per core and reassembled on host.
"""

import os
import sys

sys.path.insert(0, "/opt/trn_rl_repo")

import numpy as np

import concourse.bacc as bacc
import concourse.tile as tile
from concourse import mybir
from concourse.tile_rust import add_dep_helper

T = int(os.environ.get("RNN_T", "512"))  # env override only for debugging
B = 2048
NCORES = 8
BC = B // NCORES          # batch per core = 256
HB = BC // 2              # half-batch = 128
VOCAB, EMB, H1, H2, H3, OUT = 27, 20, 32, 64, 64, 26
S = T + 2                 # macro steps incl. pipeline flush

MM_DT = mybir.dt.bfloat16     # matmul operand dtype (states/weights)

import ml_dtypes  # noqa: E402

_NP_OF = {mybir.dt.bfloat16: ml_dtypes.bfloat16, mybir.dt.float32: np.float32}

P1 = 64   # partition base of the pre1/h1 block


def _build_nc():
    nc = bacc.Bacc()
    f32 = mybir.dt.float32
    mdt = MM_DT

    oh_d = nc.dram_tensor("oh", [VOCAB, T * BC], mdt, kind="ExternalInput")
    la_d = nc.dram_tensor("la", [H2 + H3, H2 + H3], mdt, kind="ExternalInput")
    lb_d = nc.dram_tensor("lb", [H1, H2], mdt, kind="ExternalInput")
    lc_d = nc.dram_tensor("lc", [H1, H1], mdt, kind="ExternalInput")
    le_d = nc.dram_tensor("le", [VOCAB, H1], mdt, kind="ExternalInput")
    lf_d = nc.dram_tensor("lf", [H3 + 1, OUT], mdt, kind="ExternalInput")
    b23_d = nc.dram_tensor("b23", [128, 1], f32, kind="ExternalInput")
    o_d = nc.dram_tensor("o", [OUT, T * BC], f32, kind="ExternalOutput")

    with tile.TileContext(nc) as tc:
        with (
            tc.tile_pool(name="wpool", bufs=1) as wpool,
            tc.tile_pool(name="hpool", bufs=6) as hpool,
            tc.tile_pool(name="ohpool", bufs=3) as ohpool,
            tc.tile_pool(name="h3pool", bufs=2) as h3pool,
            tc.tile_pool(name="opool", bufs=3) as opool,
            tc.tile_pool(name="ppool", bufs=4, space="PSUM") as ppool,
            tc.tile_pool(name="fcpool", bufs=2, space="PSUM") as fcpool,
            tc.tile_pool(name="warmp", bufs=1, space="PSUM") as warmp,
        ):
            la = wpool.tile([H2 + H3, H2 + H3], mdt)
            lb = wpool.tile([P1 + H1, H2], mdt)      # rows 64:96 hold W_ih2^T
            lc = wpool.tile([P1 + H1, H1], mdt)      # rows 64:96 hold W_hh1^T
            le = wpool.tile([P1 + VOCAB, H1], mdt)   # rows 64:91 hold EW''
            lf = wpool.tile([H3 + 1, OUT], mdt)
            b23 = wpool.tile([128, 1], f32)
            nc.sync.dma_start(la[:], la_d[:])
            nc.sync.dma_start(lb[P1:P1 + H1, :], lb_d[:])
            nc.sync.dma_start(lc[P1:P1 + H1, :], lc_d[:])
            nc.sync.dma_start(le[P1:P1 + VOCAB, :], le_d[:])
            nc.sync.dma_start(lf[:], lf_d[:])
            nc.sync.dma_start(b23[:], b23_d[:])

            zst = wpool.tile([128, 2 * HB], mdt)   # zero initial state
            nc.vector.memset(zst[:], 0.0)
            zoh = wpool.tile([P1 + VOCAB, HB], mdt)  # zero one-hot, flush steps
            nc.vector.memset(zoh[:], 0.0)

            # PE warmup: ~5us of back-to-back matmuls trips the HAM clock
            # gate to 8/8 (2.4 GHz) before the latency-critical loop begins.
            warm = wpool.tile([128, 512], mdt)
            nc.vector.memset(warm[:], 0.0)
            wp = warmp.tile([128, 512], mybir.dt.float32)
            for _ in range(12):
                nc.tensor.matmul(wp[:], warm[:, 0:128], warm[:], start=True, stop=True)

            # Prime all ppool PSUM slots: regions outside the four matmuls'
            # outputs are read by the packed ACT; zero them once.
            for _ in range(4):
                pp = ppool.tile([128, 2 * HB], mybir.dt.float32, tag="p")
                nc.vector.memset(pp[:], 0.0)

            # Prime h3pool slots' constant-ones row (row 64) for the fc bias.
            h3slots = []
            for _ in range(2):
                hb_ = h3pool.tile([H3 + 1, 4 * HB], mdt, tag="h3buf")
                nc.vector.memset(hb_[H3:H3 + 1, :], 1.0)
                h3slots.append(hb_)

            hprev = [zst, zst]
            oht = None
            h3buf = None
            tanh = mybir.ActivationFunctionType.Tanh

            for s in range(S):
                if s % 2 == 0 and s < T:
                    g = s // 2
                    oht = ohpool.tile([P1 + VOCAB, 4 * HB], mdt)
                    nc.sync.dma_start(oht[P1:P1 + VOCAB, :],
                                      oh_d[:, 4 * HB * g:4 * HB * (g + 1)])
                for half in range(2):
                    hp = hprev[half]
                    p = ppool.tile([128, 2 * HB], f32)
                    if s < T:
                        o0 = (s % 2) * 2 * HB + half * HB
                        ohs = oht[P1:P1 + VOCAB, o0:o0 + HB]
                    else:
                        ohs = zoh[P1:P1 + VOCAB, :]
                    # One accumulation group per half-step+bank. start=True
                    # clears has_written for the written PARTITIONS across the
                    # full bank width, so the opener must cover all 128
                    # partitions (la does); everything after accumulates (or
                    # overwrites regions whose bits the opener cleared).
                    mm_la = nc.tensor.matmul(p[:, 0:HB], la[:], hp[:, 0:HB],
                                             start=True, stop=False,
                                             skip_group_check=True)
                    # pre1 = EW''[x_t] + h1 recurrence
                    mm_e = nc.tensor.matmul(p[P1:P1 + H1, HB:2 * HB],
                                            le[P1:P1 + VOCAB, :],
                                            ohs, start=False, stop=False,
                                            tile_position=(P1, P1),
                                            skip_group_check=True)
                    add_dep_helper(mm_e.ins, mm_la.ins, sync=False,
                                   reason="group opener executes first")
                    nc.tensor.matmul(p[0:H2, 0:HB], lb[P1:P1 + H1, :],
                                     hp[P1:P1 + H1, HB:2 * HB],
                                     start=False, stop=False,
                                     tile_position=(P1, 0), skip_group_check=True)
                    nc.tensor.matmul(p[P1:P1 + H1, HB:2 * HB], lc[P1:P1 + H1, :],
                                     hp[P1:P1 + H1, HB:2 * HB],
                                     start=False, stop=True,
                                     tile_position=(P1, P1), skip_group_check=True)
                    hn = hpool.tile([128, 2 * HB], mdt)
                    nc.scalar.activation(hn[:], p[:], tanh, bias=b23[:])
                    if s == 0:
                        nc.vector.memset(hn[:, 0:HB], 0.0)       # H2, H3 invalid
                    elif s == 1:
                        nc.vector.memset(hn[H2:128, 0:HB], 0.0)  # H3 invalid
                    hprev[half] = hn
                    # collect h3 (valid output for t3 = s-2)
                    if s >= 2:
                        j = 2 * ((s - 2) % 2) + half
                        if j == 0:
                            h3buf = h3slots[((s - 2) // 2) % 2]
                        nc.vector.tensor_copy(h3buf[0:H3, HB * j:HB * (j + 1)],
                                              hn[H2:128, 0:HB])
                        if j == 3:
                            c = (s - 2) // 2
                            fco = fcpool.tile([OUT, 4 * HB], f32)
                            nc.tensor.matmul(fco[:], lf[:], h3buf[:],
                                             start=True, stop=True)
                            outs = opool.tile([OUT, 4 * HB], f32)
                            nc.vector.tensor_copy(outs[:], fco[:])
                            nc.sync.dma_start(o_d[:, 4 * HB * c:4 * HB * (c + 1)],
                                              outs[:])
    nc.compile()
    return nc


_NC_CACHE = None


def _get_nc():
    global _NC_CACHE
    if _NC_CACHE is None:
        _NC_CACHE = _build_nc()
    return _NC_CACHE


def _prep_inputs(inputs):
    npdt = _NP_OF[MM_DT]
    f32 = np.float32
    x = np.asarray(inputs["x"]).astype(np.int64)            # (T, B)
    emb = np.asarray(inputs["emb"], f32)
    W_ih1 = np.asarray(inputs["W_ih1"], f32)
    W_hh1 = np.asarray(inputs["W_hh1"], f32)
    b1 = np.asarray(inputs["b_ih1"], f32) + np.asarray(inputs["b_hh1"], f32)
    W_ih2 = np.asarray(inputs["W_ih2"], f32)
    W_hh2 = np.asarray(inputs["W_hh2"], f32)
    b2 = np.asarray(inputs["b_ih2"], f32) + np.asarray(inputs["b_hh2"], f32)
    W_ih3 = np.asarray(inputs["W_ih3"], f32)
    W_hh3 = np.asarray(inputs["W_hh3"], f32)
    b3 = np.asarray(inputs["b_ih3"], f32) + np.asarray(inputs["b_hh3"], f32)
    W_fc = np.asarray(inputs["W_fc"], f32)
    b_fc = np.asarray(inputs["b_fc"], f32)

    # lhsT blocks (stationary operands, [K, M])
    la = np.zeros((H2 + H3, H2 + H3), f32)
    la[0:H2, 0:H2] = W_hh2.T
    la[0:H2, H2:] = W_ih3.T
    la[H2:, H2:] = W_hh3.T
    lb = W_ih2.T.copy()                                      # [32, 64]
    lc = W_hh1.T.copy()                                      # [32, 32]
    # EW'' table: emb @ W_ih1^T + b1, minus the b23[64:96] (= b3[:32]) that
    # the ACT bias vector adds on the pre1 partitions.
    le = emb @ W_ih1.T + b1[None, :] - b3[None, 0:H1]        # [27, 32]
    lf = np.zeros((H3 + 1, OUT), f32)
    lf[0:H3, :] = W_fc.T
    lf[H3, :] = b_fc                                         # ones-row bias
    b23 = np.concatenate([b2, b3]).reshape(128, 1).astype(f32)

    shared = {
        "la": la.astype(npdt), "lb": lb.astype(npdt), "lc": lc.astype(npdt),
        "le": le.astype(npdt), "lf": lf.astype(npdt), "b23": b23,
    }
    in_maps = []
    for core in range(NCORES):
        xc = x[:, core * BC:(core + 1) * BC]                 # (T, BC)
        # one-hot [27, T*BC], free order (t, b)
        oh = (xc.reshape(T * BC)[None, :] == np.arange(VOCAB)[:, None])
        in_maps.append(dict(shared, oh=np.ascontiguousarray(oh.astype(npdt))))
    return in_maps


def _assemble(results):
    cores = []
    for core in range(NCORES):
        o = results[core]["o"]                               # [26, T*BC]
        cores.append(o.reshape(OUT, T, BC).transpose(1, 2, 0))
    return np.ascontiguousarray(np.concatenate(cores, axis=1), dtype=np.float32)


def _run(inputs, **spmd_kwargs):
    """Returns (output, BassKernelResults). spmd_kwargs e.g. trace=True."""
    from concourse.bass_utils import run_bass_kernel_spmd
    nc = _get_nc()
    in_maps = _prep_inputs(inputs)
    res = run_bass_kernel_spmd(nc, in_maps, core_ids=list(range(NCORES)),
                               **spmd_kwargs)
    return _assemble(res.results), res


def kernel(**inputs) -> np.ndarray:
    return _run(inputs)[0]


if __name__ == "__main__":
    import reference as R
    ins = {k: np.asarray(v) for k, v in R.setup_inputs().items()}
    got = kernel(**ins)
    import jax.numpy as jnp
    want = np.asarray(R.reference(**{k: jnp.asarray(v) for k, v in ins.items()}))
    err = np.abs(got - want)
    print("absmax", err.max(), "rel", err.max() / np.abs(want).max())

